# revision 25
# baseline (speedup 1.0000x reference)
"""Trainium2 Bass kernel for nn_GPTrack2D (dense transformer with linear
attention and a per-frame recurrence over L).

Sharding: batch (2) -> two groups of 4 cores; tokens (1024 -> 256/core)
within each group. Linear attention's k^T v state is all-reduced per frame
within the group (fp16 payload); the all-reduce and all per-frame stalls
hide behind the previous frame's MLP, which is emitted as gap-filler work
after each frame's critical ops (the Tile scheduler is out-of-order per
engine, by emission-order priority).

Precision: residual stream / carry / LN inputs are fp32. LN stats run as
one fused bf16 [x | x^2] ones-matmul per d-tile (free dim 512). rb comes
from the Scalar engine's Dsqrt (0.5/sqrt); the 0.5 is folded into the
qkv/mlp weight scales on the host. pos embeddings are folded into x on
the host (layer 0) and into the layer-0 backward MLP output (layer 1).
"""

import functools

import ml_dtypes
import numpy as np

import concourse.bacc as bacc
import concourse.mybir as mybir
from concourse import tile
from concourse.bass_utils import run_bass_kernel_spmd

F32 = mybir.dt.float32
BF16 = mybir.dt.bfloat16
F16 = mybir.dt.float16
AF = mybir.ActivationFunctionType
ALU = mybir.AluOpType

B, L, N, D, M, H = 2, 12, 1024, 768, 3072, 12
NCORES = 8
GROUP = 4                 # cores per batch group
TOK = N // GROUP          # 256 tokens per core
KT = D // 128             # 6 d-tiles
MT = M // 128             # 24 m-tiles
MJC = 2                   # m-tiles per MLP weight DMA chunk
F3 = 3 * D                # 2304
EPS = 1e-5
KVS = 1.0 / 256.0         # kv-state scale so fp16 holds it comfortably
KVSI = 256.0

# dev-scale knobs (full problem: L_RUN=12, LAYERS_RUN=2, DIRS_RUN=(0, 1))
L_RUN = L
LAYERS_RUN = 2
DIRS_RUN = (0, 1)

REPLICA_GROUPS = [[0, 1, 2, 3], [4, 5, 6, 7]]


# ---------------------------------------------------------------- host prep

def _pack_weights(inputs, dtype=np.float16):
    """Fold LN gains/biases into weights; x2 for the Dsqrt 0.5; tile for DMA."""
    segs = []
    for layer in range(LAYERS_RUN):
        for d in DIRS_RUN:
            gi = np.asarray(inputs["lni_g"][d, layer]); bi = np.asarray(inputs["lni_b"][d, layer])
            gh = np.asarray(inputs["lnh_g"][d, layer]); bh = np.asarray(inputs["lnh_b"][d, layer])
            go = np.asarray(inputs["lno_g"][d, layer]); bo = np.asarray(inputs["lno_b"][d, layer])
            Wqkv = np.asarray(inputs["Wqkv"][d, layer]); bqkv = np.asarray(inputs["bqkv"][d, layer])
            Wqkvh = np.asarray(inputs["Wqkvh"][d, layer]); bqkvh = np.asarray(inputs["bqkvh"][d, layer])
            Wout = np.asarray(inputs["Wout"][d, layer]); bout = np.asarray(inputs["bout"][d, layer])
            W1 = np.asarray(inputs["W1"][d, layer]); b1 = np.asarray(inputs["b1"][d, layer])
            W2 = np.asarray(inputs["W2"][d, layer]); b2 = np.asarray(inputs["b2"][d, layer])

            gqkv = gi[:, None] * Wqkv                      # (D, 3D)
            gqkvh = gh[:, None] * Wqkvh
            cqkv = bi @ Wqkv + bqkv + bh @ Wqkvh + bqkvh   # (3D,)
            g1 = go[:, None] * W1                          # (D, M)
            c1 = bo @ W1 + b1                              # (M,)

            seg = dict(
                # (128, KT, F3): [p, kd, f] = gqkv[kd*128+p, f]
                gqkv=np.ascontiguousarray(
                    gqkv.reshape(KT, 128, F3).transpose(1, 0, 2)).astype(dtype),
                gqkvh=np.ascontiguousarray(
                    gqkvh.reshape(KT, 128, F3).transpose(1, 0, 2)).astype(dtype),
                cqkv=cqkv.reshape(1, F3).astype(dtype),
                wout=np.ascontiguousarray(
                    Wout.reshape(KT, 128, D).transpose(1, 0, 2)).astype(dtype),
                bout=np.ascontiguousarray(
                    bout.reshape(KT, 128).T).astype(np.float32),  # (128, KT)
                # (MT//MJC, 128, MJC, KT, 128):
                #   [c, p, j, kd, f] = g1[kd*128+p, (c*MJC+j)*128+f]
                g1=np.ascontiguousarray(
                    g1.reshape(KT, 128, MT // MJC, MJC, 128)
                    .transpose(2, 1, 3, 0, 4)).astype(dtype),
                c1=np.ascontiguousarray(
                    c1.reshape(MT, 128).T).astype(np.float32),    # (128, MT)
                # (MT//MJC, 128, MJC, D): [c, p, j, f] = W2[(c*MJC+j)*128+p, f]
                w2=np.ascontiguousarray(
                    W2.reshape(MT // MJC, MJC, 128, D)
                    .transpose(0, 2, 1, 3)).astype(dtype),
                b2=np.ascontiguousarray(
                    b2.reshape(KT, 128).T).astype(np.float32),    # (128, KT)
            )
            segs.append(seg)
    return segs


def _feat_major(a, dtype):
    """(..., tok, D) -> (..., 128, KT, tok) tiled feature-major."""
    t = np.moveaxis(np.asarray(a), -1, -2)                # (..., D, tok)
    shp = t.shape[:-2]
    t = t.reshape(shp + (KT, 128, t.shape[-1]))           # (..., KT, 128, tok)
    t = np.moveaxis(t, -3, -2)                            # (..., 128, KT, tok)
    return np.ascontiguousarray(t).astype(dtype)


def make_in_maps(inputs):
    segs = _pack_weights(inputs)
    x = np.asarray(inputs["x"], np.float32)
    tp = np.asarray(inputs["temporal_pos"], np.float32)
    sp = np.asarray(inputs["spatial_pos"], np.float32)
    pos = tp[:, :, None, :] * sp[:, None, :, :]           # (B, L, N, D)
    xp = x[:, :L_RUN] + pos[:, :L_RUN]                    # layer-0 x + pos fold
    in_maps = []
    for core in range(NCORES):
        b = core // GROUP
        s = (core % GROUP) * TOK
        m = {}
        m["x_in"] = _feat_major(xp[b, :, s:s + TOK, :], np.float32)
        m["h0_in"] = _feat_major(
            np.asarray(inputs["hidden"])[b, s:s + TOK, :], np.float32)
        m["pos_in"] = _feat_major(
            pos[b, :L_RUN, s:s + TOK, :], ml_dtypes.bfloat16)
        for si, seg in enumerate(segs):
            for k, v in seg.items():
                m[f"{k}_{si}"] = v
        in_maps.append(m)
    return in_maps


def unshard_output(results):
    """results: per-core dicts with 'out_x' (L_RUN, 128, KT, TOK) f32."""
    out = np.empty((B, L_RUN, N, D), np.float32)
    for core in range(NCORES):
        b = core // GROUP
        s = (core % GROUP) * TOK
        o = np.asarray(results[core]["out_x"])            # (L, 128, KT, TOK)
        o = o.transpose(0, 2, 1, 3).reshape(L_RUN, D, TOK)
        out[b, :, s:s + TOK, :] = np.moveaxis(o, -1, -2)
    return out


# ---------------------------------------------------------------- kernel build

class Ctx:
    """Pools, constants and persistent tiles used during emission."""


def _layer_norm(nc, cx, src32, tag):
    """Feature-major LN stats for an SBUF (128, KT, TOK) f32 tile.

    Stages [x | x^2] into one bf16 (128, KT, 2, TOK) tile (1 copy + 1
    square, full width), then 6 free-512 ones-matmuls give s1|s2 fused.
    Returns (rb, mrb) f32 (128, TOK): z = src*rb - mrb, where rb is
    0.5/std (Dsqrt) -- the 2x is folded into consumer weights.
    """
    xsq = cx.act.tile([128, KT, 2, TOK], BF16, name="xsq", tag="xsq")
    nc.gpsimd.tensor_copy(xsq[:, :, 0, :], src32[:])
    nc.scalar.activation(xsq[:, :, 1, :], src32[:], AF.Square)
    ps = cx.psA.tile([128, 2 * TOK], F32, name="ps", tag="ps")
    for kd in range(KT):
        nc.tensor.matmul(ps[:], cx.onesB[:], xsq[:, kd, :, :],
                         start=(kd == 0), stop=(kd == KT - 1))
    ms = cx.tmp1.tile([128, 2 * TOK], F32, name="ms", tag="ms")
    nc.vector.tensor_scalar_mul(ms[:], ps[:], 1.0 / D)
    mean, m2 = ms[:, 0:TOK], ms[:, TOK:2 * TOK]
    msq = cx.tmp1.tile([128, TOK], F32, name="msq", tag="msq")
    nc.vector.tensor_mul(msq[:], mean, mean)
    ve = cx.tmp1.tile([128, TOK], F32, name="ve", tag="ve")
    nc.vector.tensor_sub(ve[:], m2, msq[:])
    # rb = 1/sqrt(ve+eps) = exp(-0.5*ln(ve+eps)); ln+exp share one ACT table
    lnv = cx.tmp1.tile([128, TOK], F32, name="lnv", tag="lnv")
    nc.scalar.activation(lnv[:], ve[:], AF.Ln, bias=cx.epsc[:])
    rbp = cx.tmp if tag == "x" else cx.tmp1
    rb = rbp.tile([128, TOK], F32, name=f"rb_{tag}", tag=f"rb_{tag}")
    nc.scalar.activation(rb[:], lnv[:], AF.Exp, scale=-0.5)
    mrb = rbp.tile([128, TOK], F32, name=f"mrb_{tag}", tag=f"mrb_{tag}")
    nc.vector.tensor_mul(mrb[:], mean, rb[:])
    return rb, mrb


def _normalize(nc, cx, pool, src32, rb, mrb, tag):
    """z = src*rb - mrb -> fp16 (128, KT, TOK), two full-width DVE ops."""
    z = pool.tile([128, KT, TOK], F16, name=f"z_{tag}", tag=f"z_{tag}")
    rbb = rb[:].unsqueeze(1).broadcast_to([128, KT, TOK])
    mrbb = mrb[:].unsqueeze(1).broadcast_to([128, KT, TOK])
    nc.vector.tensor_mul(z[:], src32[:], rbb)
    nc.vector.tensor_sub(z[:], z[:], mrbb)
    return z


def _elu1(nc, cx, psum_ap, out_ap, ncols):
    """out = elu(psum)+1 = exp(min(x,0)) + max(x,0)."""
    tmin = cx.tmp.tile([128, 512], F32, name="emin", tag="emin")
    texp = cx.tmp.tile([128, 512], F32, name="eexp", tag="eexp")
    nc.vector.tensor_scalar_min(tmin[:, :ncols], psum_ap, 0.0)
    nc.scalar.activation(texp[:, :ncols], tmin[:, :ncols], AF.Exp)
    nc.vector.scalar_tensor_tensor(out_ap, psum_ap, 0.0, texp[:, :ncols],
                                   op0=ALU.max, op1=ALU.add)


def build_nc():
    nc = bacc.Bacc("TRN2", target_bir_lowering=False, debug=False,
                   num_devices=NCORES)

    x_in = nc.dram_tensor("x_in", [L_RUN, 128, KT, TOK], F32, kind="ExternalInput")
    h0_in = nc.dram_tensor("h0_in", [128, KT, TOK], F32, kind="ExternalInput")
    pos_in = nc.dram_tensor("pos_in", [L_RUN, 128, KT, TOK], BF16, kind="ExternalInput")
    nseg = LAYERS_RUN * len(DIRS_RUN)
    segs = []
    for si in range(nseg):
        segs.append(dict(
            gqkv=nc.dram_tensor(f"gqkv_{si}", [128, KT, F3], F16, kind="ExternalInput"),
            gqkvh=nc.dram_tensor(f"gqkvh_{si}", [128, KT, F3], F16, kind="ExternalInput"),
            cqkv=nc.dram_tensor(f"cqkv_{si}", [1, F3], F16, kind="ExternalInput"),
            wout=nc.dram_tensor(f"wout_{si}", [128, KT, D], F16, kind="ExternalInput"),
            bout=nc.dram_tensor(f"bout_{si}", [128, KT], F32, kind="ExternalInput"),
            g1=nc.dram_tensor(f"g1_{si}", [MT // MJC, 128, MJC, KT, 128], F16,
                              kind="ExternalInput"),
            c1=nc.dram_tensor(f"c1_{si}", [128, MT], F32, kind="ExternalInput"),
            w2=nc.dram_tensor(f"w2_{si}", [MT // MJC, 128, MJC, D], F16,
                              kind="ExternalInput"),
            b2=nc.dram_tensor(f"b2_{si}", [128, KT], F32, kind="ExternalInput"),
        ))
    out_x = nc.dram_tensor("out_x", [L_RUN, 128, KT, TOK], F32, kind="ExternalOutput")

    with tile.TileContext(nc) as tc:
        with (
            tc.tile_pool(name="cst", bufs=1) as cst,
            tc.tile_pool(name="wt", bufs=1) as wt,
            tc.tile_pool(name="wts", bufs=2) as wts,
            tc.tile_pool(name="stream", bufs=3) as stream,
            tc.tile_pool(name="y1p", bufs=4) as y1p,
            tc.tile_pool(name="act", bufs=2) as actp,
            tc.tile_pool(name="actx", bufs=2) as actx,
            tc.tile_pool(name="act1", bufs=1) as act1,
            tc.tile_pool(name="state", bufs=1) as state,
            tc.tile_pool(name="tmp", bufs=2) as tmp,
            tc.tile_pool(name="tmp1", bufs=1) as tmp1,
            tc.tile_pool(name="psA", bufs=5, space="PSUM") as psA,
            tc.tile_pool(name="psY", bufs=3, space="PSUM") as psY,
            tc.tile_pool(name="dram", bufs=4, space="DRAM") as dram,
        ):
            cx = Ctx()
            cx.wt, cx.wts, cx.stream, cx.y1p = wt, wts, stream, y1p
            cx.act, cx.actx, cx.act1 = actp, actx, act1
            cx.state, cx.tmp, cx.tmp1 = state, tmp, tmp1
            cx.psA, cx.psY, cx.dram = psA, psY, dram

            cx.onesB = cst.tile([128, 128], BF16, name="onesB")
            nc.vector.memset(cx.onesB[:], 1.0)
            cx.ones1 = cst.tile([1, TOK], F16, name="ones1")
            nc.vector.memset(cx.ones1[:], 1.0)
            cx.epsc = cst.tile([128, 1], F32, name="epsc")
            nc.vector.memset(cx.epsc[:], EPS)
            # block-diag kv holder: off-diagonal blocks stay zero forever
            cx.bd16 = state.tile([128, KT, 128], F16, name="bd16", tag="bd16")
            nc.vector.memset(cx.bd16[:], 0.0)

            x1_sc = dram.tile([L_RUN, 128, KT, TOK], F32, name="x1_sc", tag="x1_sc")
            yf_sc = dram.tile([L_RUN, 128, KT, TOK], F32, name="yf_sc", tag="yf_sc")

            pend = None
            for layer in range(LAYERS_RUN):
                x_src = x_in.ap() if layer == 0 else x1_sc
                last_layer = layer == LAYERS_RUN - 1
                for dir_i, d in enumerate(DIRS_RUN):
                    si = layer * len(DIRS_RUN) + dir_i
                    fwd = d == 0
                    last_scan = dir_i == len(DIRS_RUN) - 1
                    frames = (list(range(L_RUN)) if fwd
                              else list(range(L_RUN - 1, -1, -1)))
                    if not last_scan:
                        out_dst = yf_sc
                    elif last_layer:
                        out_dst = out_x.ap()
                    else:
                        out_dst = x1_sc
                    # L0-bwd MLP adds pos into x1 so layer 1 loads it folded
                    add_pos = (out_dst is x1_sc)
                    pend = _emit_scan(nc, cx, segs[si], x_src, h0_in, pos_in,
                                      frames, fwd=fwd, layer=layer,
                                      yf_sc=yf_sc, out_dst=out_dst,
                                      add_pos=add_pos, pend=pend)
            _emit_mlp(nc, cx, pend)
    nc.compile()
    return nc


def _emit_scan(nc, cx, seg, x_src, h0_in, pos_in, frames, fwd, layer,
               yf_sc, out_dst, add_pos, pend):
    w = {}
    for nm, shape, dt in (("gqkv", [128, KT, F3], F16),
                          ("gqkvh", [128, KT, F3], F16),
                          ("wout", [128, KT, D], F16),
                          ("cqkv", [1, F3], F16)):
        w[nm] = cx.wt.tile(shape, dt, name=nm, tag=nm)
        nc.sync.dma_start(w[nm][:], seg[nm].ap())
    for nm, shape in (("bout", [128, KT]), ("c1", [128, MT]), ("b2", [128, KT])):
        w[nm] = cx.wts.tile(shape, F32, name=nm, tag=nm)
        nc.sync.dma_start(w[nm][:], seg[nm].ap())

    # h carry (f32), re-initialized from h0 each scan
    h32 = cx.state.tile([128, KT, TOK], F32, name="h32", tag="h32")
    nc.sync.dma_start(h32[:], h0_in.ap())

    # fwd: h gets pos[layer] every frame (fixed); bwd: pos[t] per frame
    post_h = None
    if fwd:
        post_h = cx.act1.tile([128, KT, TOK], BF16, name="post_h", tag="post_h")
        nc.sync.dma_start(post_h[:], pos_in.ap()[layer])

    for t in frames:
        pend = _emit_frame(nc, cx, seg, w, t, x_src, h32, pos_in, post_h,
                           yf_sc, fwd, out_dst, add_pos, pend)
    return pend


def _emit_frame(nc, cx, seg, w, t, x_src, h32, pos_in, post_h, yf_sc, fwd,
                out_dst, add_pos, pend):
    # segment-crossing MLP must be emitted before this frame's x DMA
    # (x1_sc RAW follows emission order); in-segment MLP goes at the end
    # so this frame's critical ops outrank it in scheduler priority.
    if pend is not None and pend["w"] is not w:
        _emit_mlp(nc, cx, pend)
        pend = None

    # ---- load x_t (pos already folded in); h_eff = h + pos
    xeff = cx.actx.tile([128, KT, TOK], F32, name="xe", tag="xe")
    nc.sync.dma_start(xeff[:], x_src[t])
    if fwd:
        postile = post_h
    else:
        postile = cx.actx.tile([128, KT, TOK], BF16, name="post", tag="post")
        nc.sync.dma_start(postile[:], pos_in.ap()[t])
    heff = cx.act1.tile([128, KT, TOK], F32, name="heff", tag="heff")
    nc.vector.tensor_add(heff[:], h32[:], postile[:])

    # ---- layer norms + normalized activations (fp16)
    rb_h, mrb_h = _layer_norm(nc, cx, heff, "h")
    zh = _normalize(nc, cx, cx.act1, heff, rb_h, mrb_h, "h")
    rb_x, mrb_x = _layer_norm(nc, cx, xeff, "x")
    zx = _normalize(nc, cx, cx.actx, xeff, rb_x, mrb_x, "x")

    # ---- k, v (token-major): (128, 2, D) each [tok-half, feature]
    # three 512-wide chunks over the 1536 k|v columns per token-half
    k16 = cx.act1.tile([128, 2, D], F16, name="k16", tag="k16")
    v16 = cx.act1.tile([128, 2, D], F16, name="v16", tag="v16")
    for tok2 in range(2):
        for ch in range(3):
            lo = D + ch * 512
            ps = cx.psA.tile([128, 2 * TOK], F32, name="ps", tag="ps")
            pc = ps[:, 0:512]
            for kd in range(KT):
                nc.tensor.matmul(pc, zx[:, kd, tok2 * 128:(tok2 + 1) * 128],
                                 w["gqkv"][:, kd, lo:lo + 512],
                                 start=(kd == 0), stop=False)
            for kd in range(KT):
                nc.tensor.matmul(pc, zh[:, kd, tok2 * 128:(tok2 + 1) * 128],
                                 w["gqkvh"][:, kd, lo:lo + 512],
                                 start=False, stop=False)
            nc.tensor.matmul(pc, cx.ones1[:, 0:128],
                             w["cqkv"][:, lo:lo + 512], start=False, stop=True)
            off = ch * 512
            if ch == 0:
                _elu1(nc, cx, pc, k16[:, tok2, 0:512], 512)
            elif ch == 1:
                _elu1(nc, cx, ps[:, 0:256], k16[:, tok2, 512:768], 256)
                nc.scalar.activation(v16[:, tok2, 0:256], ps[:, 256:512], AF.Copy)
            else:
                nc.scalar.activation(v16[:, tok2, 256:768], pc, AF.Copy)

    # ---- kv state per head-pair; two psums of 3 head-pairs each, then
    # pack diag blocks into (128, 384) f16 with strided DVE ops
    kvpack = cx.act1.tile([128, H * 32], F16, name="kvpack", tag="kvpack")
    for g in range(2):
        ps = cx.psA.tile([128, 2 * TOK], F32, name="ps", tag="ps")
        for i in range(3):
            hp = g * 3 + i
            pskv = ps[:, i * 128:(i + 1) * 128]
            for tok2 in range(2):
                nc.tensor.matmul(pskv, k16[:, tok2, hp * 128:(hp + 1) * 128],
                                 v16[:, tok2, hp * 128:(hp + 1) * 128],
                                 start=(tok2 == 0), stop=(tok2 == 1))
        for half in range(2):
            pr = slice(half * 64, half * 64 + 64)
            src = ps[pr, half * 64:384 + half * 64].rearrange(
                "p (i c) -> p i c", i=3)[:, :, 0:64]
            nc.vector.tensor_scalar_mul(
                kvpack[pr, g * 192:(g + 1) * 192].rearrange(
                    "p (i c) -> p i c", i=3),
                src, KVS)

    # ---- all-reduce kv (fp16) within the token-shard group
    arin = cx.dram.tile([128, H * 32], F16, name="arin", tag="arin")
    arout = cx.dram.tile([128, H * 32], F16, name="arout", tag="arout")
    nc.sync.dma_start(arin[:], kvpack[:])
    nc.gpsimd.collective_compute(
        "AllReduce", ALU.add, replica_groups=REPLICA_GROUPS,
        ins=[arin.opt()], outs=[arout.opt()])

    # ---- q (feature-major): consumed only after the AR, so its matmuls
    # naturally cover the all-reduce latency window
    q16 = cx.act1.tile([128, KT, TOK], F16, name="q16", tag="q16")
    for ft in range(KT):
        ps = cx.psA.tile([128, 2 * TOK], F32, name="ps", tag="ps")
        pq = ps[:, 0:TOK]
        for kd in range(KT):
            nc.tensor.matmul(pq, w["gqkv"][:, kd, ft * 128:(ft + 1) * 128],
                             zx[:, kd, :], start=(kd == 0), stop=False)
        for kd in range(KT):
            nc.tensor.matmul(pq, w["gqkvh"][:, kd, ft * 128:(ft + 1) * 128],
                             zh[:, kd, :], start=False, stop=False)
        nc.tensor.matmul(pq, w["cqkv"][:, ft * 128:(ft + 1) * 128],
                         cx.ones1[:], start=False, stop=True)
        _elu1(nc, cx, pq, q16[:, ft, :], TOK)

    kvred = cx.act1.tile([128, H * 32], F16, name="kvred", tag="kvred")
    nc.sync.dma_start(kvred[:], arout[:])

    # ---- block-diag kv; o_s = blockdiag(kv_s) @ q   (carries KVS)
    for half in range(2):
        pr = slice(half * 64, half * 64 + 64)
        nc.gpsimd.tensor_copy(
            cx.bd16[pr, :, half * 64:half * 64 + 64],
            kvred[pr, :].rearrange("p (k c) -> p k c", k=KT))
    o16 = cx.act1.tile([128, KT, TOK], F16, name="o16", tag="o16")
    for hp in range(KT):
        ps = cx.psA.tile([128, 2 * TOK], F32, name="ps", tag="ps")
        nc.tensor.matmul(ps[:, 0:TOK], cx.bd16[:, hp, :], q16[:, hp, :],
                         start=True, stop=True)
        nc.scalar.activation(o16[:, hp, :], ps[:, 0:TOK], AF.Copy)

    # ---- attn out; h_next = attn + heff; x2 = attn + xeff (in place)
    at_all = cx.act1.tile([128, KT, TOK], F32, name="at_all", tag="at_all")
    for ft in range(KT):
        ps = cx.psA.tile([128, 2 * TOK], F32, name="ps", tag="ps")
        pa = ps[:, 0:TOK]
        for hp in range(KT):
            nc.tensor.matmul(pa, w["wout"][:, hp, ft * 128:(ft + 1) * 128],
                             o16[:, hp, :], start=(hp == 0), stop=(hp == KT - 1))
        # attn = ps*256 + bout  (matmul carries KVS)
        nc.scalar.activation(at_all[:, ft, :], pa, AF.Identity,
                             bias=w["bout"][:, ft:ft + 1], scale=KVSI)
    nc.vector.tensor_add(h32[:], at_all[:], heff[:])
    nc.gpsimd.tensor_add(xeff[:], at_all[:], xeff[:])   # xeff becomes x2

    # ---- z2 for the deferred MLP
    rb2, mrb2 = _layer_norm(nc, cx, xeff, "o")
    z2 = _normalize(nc, cx, cx.actx, xeff, rb2, mrb2, "o")

    new_pend = dict(t=t, z2=z2, x232=xeff, fwd=fwd, out_dst=out_dst,
                    yf_sc=yf_sc, add_pos=add_pos, postile=postile,
                    seg=seg, w=w)
    # in-segment MLP of the previous frame: emitted last so this frame's
    # critical chain outranks it; it still fills every PE idle window
    if pend is not None:
        _emit_mlp(nc, cx, pend)
    return new_pend


def _emit_mlp(nc, cx, pend):
    t, z2, x232 = pend["t"], pend["z2"], pend["x232"]
    fwd, out_dst, yf_sc = pend["fwd"], pend["out_dst"], pend["yf_sc"]
    seg, w = pend["seg"], pend["w"]

    # y1 = gelu(z2 @ G1 + c1) per m-tile, immediately consumed by W2
    yps = [cx.psY.tile([128, 2 * TOK], F32, name="psy", tag="psy")
           for _ in range(KT // 2)]
    for c in range(MT // MJC):
        g1s = cx.stream.tile([128, MJC, KT, 128], F16, name="g1s", tag="g1s")
        nc.sync.dma_start(g1s[:], seg["g1"].ap()[c])
        w2s = cx.stream.tile([128, MJC, D], F16, name="w2s", tag="w2s")
        nc.sync.dma_start(w2s[:], seg["w2"].ap()[c])
        for j in range(MJC):
            mj = c * MJC + j
            ps = cx.psA.tile([128, 2 * TOK], F32, name="ps", tag="ps")
            pj = ps[:, 0:TOK]
            for kd in range(KT):
                nc.tensor.matmul(pj, g1s[:, j, kd, :], z2[:, kd, :],
                                 start=(kd == 0), stop=(kd == KT - 1))
            y1s = cx.y1p.tile([128, TOK], F16, name="y1s", tag="y1s")
            nc.scalar.activation(y1s[:], pj, AF.Gelu,
                                 bias=w["c1"][:, mj:mj + 1])
            for ft in range(KT):
                nc.tensor.matmul(
                    yps[ft // 2][:, (ft % 2) * TOK:(ft % 2 + 1) * TOK],
                    w2s[:, j, ft * 128:(ft + 1) * 128], y1s[:],
                    start=(mj == 0), stop=(mj == MT - 1))

    outt = cx.act1.tile([128, KT, TOK], F32, name="outt", tag="outt")
    for ft in range(KT):
        nc.vector.scalar_tensor_tensor(
            outt[:, ft, :],
            yps[ft // 2][:, (ft % 2) * TOK:(ft % 2 + 1) * TOK],
            w["b2"][:, ft:ft + 1], x232[:, ft, :],
            op0=ALU.add, op1=ALU.add)
    if not fwd:
        yf = cx.act1.tile([128, KT, TOK], F32, name="yfld", tag="yfld")
        nc.sync.dma_start(yf[:], yf_sc[t])
        nc.gpsimd.tensor_add(outt[:], outt[:], yf[:])
        if pend["add_pos"]:
            nc.gpsimd.tensor_add(outt[:], outt[:], pend["postile"][:])
    nc.sync.dma_start(out_dst[t], outt[:])


# ---------------------------------------------------------------- entry point

@functools.cache
def _compiled_nc():
    return build_nc()


def kernel(**inputs):
    inputs = {k: np.asarray(v) for k, v in inputs.items()}
    nc = _compiled_nc()
    in_maps = make_in_maps(inputs)
    res = run_bass_kernel_spmd(nc, in_maps, list(range(NCORES)))
    return unshard_output(res.results)


# revision 28
# speedup vs baseline: 1.0821x; 1.0821x over previous
"""Trainium2 Bass kernel for nn_GPTrack2D (dense transformer with linear
attention and a per-frame recurrence over L).

Sharding: batch (2) -> two groups of 4 cores; tokens (1024 -> 256/core)
within each group. Linear attention's k^T v state is all-reduced per frame
within the group (fp16 payload); the all-reduce and all per-frame stalls
hide behind the previous frame's MLP, which is emitted as gap-filler work
after each frame's critical ops (the Tile scheduler is out-of-order per
engine, by emission-order priority).

Precision: residual stream / carry / LN inputs are fp32. LN stats run as
one fused bf16 [x | x^2] ones-matmul per d-tile (free dim 512). rb comes
from the Scalar engine's Dsqrt (0.5/sqrt); the 0.5 is folded into the
qkv/mlp weight scales on the host. pos embeddings are folded into x on
the host (layer 0) and into the layer-0 backward MLP output (layer 1).
"""

import functools

import ml_dtypes
import numpy as np

import concourse.bacc as bacc
import concourse.mybir as mybir
from concourse import tile
from concourse.bass_utils import run_bass_kernel_spmd

F32 = mybir.dt.float32
BF16 = mybir.dt.bfloat16
F16 = mybir.dt.float16
AF = mybir.ActivationFunctionType
ALU = mybir.AluOpType

B, L, N, D, M, H = 2, 12, 1024, 768, 3072, 12
NCORES = 8
GROUP = 4                 # cores per batch group
TOK = N // GROUP          # 256 tokens per core
KT = D // 128             # 6 d-tiles
MT = M // 128             # 24 m-tiles
MJC = 2                   # m-tiles per MLP weight DMA chunk
F3 = 3 * D                # 2304
EPS = 1e-5
KVS = 1.0 / 256.0         # kv-state scale so fp16 holds it comfortably
KVSI = 256.0

# dev-scale knobs (full problem: L_RUN=12, LAYERS_RUN=2, DIRS_RUN=(0, 1))
L_RUN = L
LAYERS_RUN = 2
DIRS_RUN = (0, 1)

REPLICA_GROUPS = [[0, 1, 2, 3], [4, 5, 6, 7]]


# ---------------------------------------------------------------- host prep

def _pack_weights(inputs, dtype=np.float16):
    """Fold LN gains/biases into weights; x2 for the Dsqrt 0.5; tile for DMA."""
    segs = []
    for layer in range(LAYERS_RUN):
        for d in DIRS_RUN:
            gi = np.asarray(inputs["lni_g"][d, layer]); bi = np.asarray(inputs["lni_b"][d, layer])
            gh = np.asarray(inputs["lnh_g"][d, layer]); bh = np.asarray(inputs["lnh_b"][d, layer])
            go = np.asarray(inputs["lno_g"][d, layer]); bo = np.asarray(inputs["lno_b"][d, layer])
            Wqkv = np.asarray(inputs["Wqkv"][d, layer]); bqkv = np.asarray(inputs["bqkv"][d, layer])
            Wqkvh = np.asarray(inputs["Wqkvh"][d, layer]); bqkvh = np.asarray(inputs["bqkvh"][d, layer])
            Wout = np.asarray(inputs["Wout"][d, layer]); bout = np.asarray(inputs["bout"][d, layer])
            W1 = np.asarray(inputs["W1"][d, layer]); b1 = np.asarray(inputs["b1"][d, layer])
            W2 = np.asarray(inputs["W2"][d, layer]); b2 = np.asarray(inputs["b2"][d, layer])

            gqkv = gi[:, None] * Wqkv                      # (D, 3D)
            gqkvh = gh[:, None] * Wqkvh
            cqkv = bi @ Wqkv + bqkv + bh @ Wqkvh + bqkvh   # (3D,)
            g1 = go[:, None] * W1                          # (D, M)
            c1 = bo @ W1 + b1                              # (M,)

            seg = dict(
                # (128, KT, F3): [p, kd, f] = gqkv[kd*128+p, f]
                gqkv=np.ascontiguousarray(
                    gqkv.reshape(KT, 128, F3).transpose(1, 0, 2)).astype(dtype),
                gqkvh=np.ascontiguousarray(
                    gqkvh.reshape(KT, 128, F3).transpose(1, 0, 2)).astype(dtype),
                cqkv=cqkv.reshape(1, F3).astype(dtype),
                wout=np.ascontiguousarray(
                    Wout.reshape(KT, 128, D).transpose(1, 0, 2)).astype(dtype),
                bout=np.ascontiguousarray(
                    bout.reshape(KT, 128).T).astype(np.float32),  # (128, KT)
                # (MT//MJC, 128, MJC, KT, 128):
                #   [c, p, j, kd, f] = g1[kd*128+p, (c*MJC+j)*128+f]
                g1=np.ascontiguousarray(
                    g1.reshape(KT, 128, MT // MJC, MJC, 128)
                    .transpose(2, 1, 3, 0, 4)).astype(dtype),
                c1=np.ascontiguousarray(
                    c1.reshape(MT, 128).T).astype(np.float32),    # (128, MT)
                # (MT//MJC, 128, MJC, D): [c, p, j, f] = W2[(c*MJC+j)*128+p, f]
                w2=np.ascontiguousarray(
                    W2.reshape(MT // MJC, MJC, 128, D)
                    .transpose(0, 2, 1, 3)).astype(dtype),
                b2=np.ascontiguousarray(
                    b2.reshape(KT, 128).T).astype(np.float32),    # (128, KT)
            )
            segs.append(seg)
    return segs


def _feat_major(a, dtype):
    """(..., tok, D) -> (..., 128, KT, tok) tiled feature-major."""
    t = np.moveaxis(np.asarray(a), -1, -2)                # (..., D, tok)
    shp = t.shape[:-2]
    t = t.reshape(shp + (KT, 128, t.shape[-1]))           # (..., KT, 128, tok)
    t = np.moveaxis(t, -3, -2)                            # (..., 128, KT, tok)
    return np.ascontiguousarray(t).astype(dtype)


def make_in_maps(inputs):
    segs = _pack_weights(inputs)
    x = np.asarray(inputs["x"], np.float32)
    tp = np.asarray(inputs["temporal_pos"], np.float32)
    sp = np.asarray(inputs["spatial_pos"], np.float32)
    pos = tp[:, :, None, :] * sp[:, None, :, :]           # (B, L, N, D)
    xp = x[:, :L_RUN] + pos[:, :L_RUN]                    # layer-0 x + pos fold
    in_maps = []
    for core in range(NCORES):
        b = core // GROUP
        s = (core % GROUP) * TOK
        m = {}
        m["x_in"] = _feat_major(xp[b, :, s:s + TOK, :], np.float32)
        m["h0_in"] = _feat_major(
            np.asarray(inputs["hidden"])[b, s:s + TOK, :], np.float32)
        m["pos_in"] = _feat_major(
            pos[b, :L_RUN, s:s + TOK, :], ml_dtypes.bfloat16)
        for si, seg in enumerate(segs):
            for k, v in seg.items():
                m[f"{k}_{si}"] = v
        in_maps.append(m)
    return in_maps


def unshard_output(results):
    """results: per-core dicts with 'out_x' (L_RUN, 128, KT, TOK) f32."""
    out = np.empty((B, L_RUN, N, D), np.float32)
    for core in range(NCORES):
        b = core // GROUP
        s = (core % GROUP) * TOK
        o = np.asarray(results[core]["out_x"])            # (L, 128, KT, TOK)
        o = o.transpose(0, 2, 1, 3).reshape(L_RUN, D, TOK)
        out[b, :, s:s + TOK, :] = np.moveaxis(o, -1, -2)
    return out


# ---------------------------------------------------------------- kernel build

class Ctx:
    """Pools, constants and persistent tiles used during emission."""


def _layer_norm(nc, cx, src32, tag):
    """Feature-major LN stats for an SBUF (128, KT, TOK) f32 tile.

    Stages [x | x^2] into one bf16 (128, KT, 2, TOK) tile (1 copy + 1
    square, full width), then 6 free-512 ones-matmuls give s1|s2 fused.
    Returns (rb, mrb) f32 (128, TOK): z = src*rb - mrb, where rb is
    0.5/std (Dsqrt) -- the 2x is folded into consumer weights.
    """
    xsq = cx.act.tile([128, KT, 2, TOK], BF16, name="xsq", tag="xsq")
    nc.vector.tensor_copy(xsq[:, :, 0, :], src32[:])
    nc.scalar.activation(xsq[:, :, 1, :], src32[:], AF.Square)
    ps = cx.psA.tile([128, 2 * TOK], F32, name="ps", tag="ps")
    for kd in range(KT):
        nc.tensor.matmul(ps[:], cx.onesB[:], xsq[:, kd, :, :],
                         start=(kd == 0), stop=(kd == KT - 1))
    ms = cx.tmp1.tile([128, 2 * TOK], F32, name="ms", tag="ms")
    nc.vector.tensor_scalar_mul(ms[:], ps[:], 1.0 / D)
    mean, m2 = ms[:, 0:TOK], ms[:, TOK:2 * TOK]
    msq = cx.tmp1.tile([128, TOK], F32, name="msq", tag="msq")
    nc.vector.tensor_mul(msq[:], mean, mean)
    ve = cx.tmp1.tile([128, TOK], F32, name="ve", tag="ve")
    nc.vector.tensor_sub(ve[:], m2, msq[:])
    # rb = 1/sqrt(ve+eps) = exp(-0.5*ln(ve+eps)); ln+exp share one ACT table
    lnv = cx.tmp1.tile([128, TOK], F32, name="lnv", tag="lnv")
    nc.scalar.activation(lnv[:], ve[:], AF.Ln, bias=cx.epsc[:])
    rbp = cx.tmp if tag == "x" else cx.tmp1
    rb = rbp.tile([128, TOK], F32, name=f"rb_{tag}", tag=f"rb_{tag}")
    nc.scalar.activation(rb[:], lnv[:], AF.Exp, scale=-0.5)
    mrb = rbp.tile([128, TOK], F32, name=f"mrb_{tag}", tag=f"mrb_{tag}")
    nc.vector.tensor_mul(mrb[:], mean, rb[:])
    return rb, mrb


def _normalize(nc, cx, pool, src32, rb, mrb, tag):
    """z = src*rb - mrb -> fp16 (128, KT, TOK), two full-width DVE ops."""
    z = pool.tile([128, KT, TOK], F16, name=f"z_{tag}", tag=f"z_{tag}")
    rbb = rb[:].unsqueeze(1).broadcast_to([128, KT, TOK])
    mrbb = mrb[:].unsqueeze(1).broadcast_to([128, KT, TOK])
    nc.vector.tensor_mul(z[:], src32[:], rbb)
    nc.vector.tensor_sub(z[:], z[:], mrbb)
    return z


def _elu1(nc, cx, psum_ap, out_ap, ncols):
    """out = elu(psum)+1 = exp(min(x,0)) + max(x,0)."""
    tmin = cx.tmp.tile([128, 512], F32, name="emin", tag="emin")
    texp = cx.tmp.tile([128, 512], F32, name="eexp", tag="eexp")
    nc.vector.tensor_scalar_min(tmin[:, :ncols], psum_ap, 0.0)
    nc.scalar.activation(texp[:, :ncols], tmin[:, :ncols], AF.Exp)
    nc.vector.scalar_tensor_tensor(out_ap, psum_ap, 0.0, texp[:, :ncols],
                                   op0=ALU.max, op1=ALU.add)


def build_nc():
    nc = bacc.Bacc("TRN2", target_bir_lowering=False, debug=False,
                   num_devices=NCORES)

    x_in = nc.dram_tensor("x_in", [L_RUN, 128, KT, TOK], F32, kind="ExternalInput")
    h0_in = nc.dram_tensor("h0_in", [128, KT, TOK], F32, kind="ExternalInput")
    pos_in = nc.dram_tensor("pos_in", [L_RUN, 128, KT, TOK], BF16, kind="ExternalInput")
    nseg = LAYERS_RUN * len(DIRS_RUN)
    segs = []
    for si in range(nseg):
        segs.append(dict(
            gqkv=nc.dram_tensor(f"gqkv_{si}", [128, KT, F3], F16, kind="ExternalInput"),
            gqkvh=nc.dram_tensor(f"gqkvh_{si}", [128, KT, F3], F16, kind="ExternalInput"),
            cqkv=nc.dram_tensor(f"cqkv_{si}", [1, F3], F16, kind="ExternalInput"),
            wout=nc.dram_tensor(f"wout_{si}", [128, KT, D], F16, kind="ExternalInput"),
            bout=nc.dram_tensor(f"bout_{si}", [128, KT], F32, kind="ExternalInput"),
            g1=nc.dram_tensor(f"g1_{si}", [MT // MJC, 128, MJC, KT, 128], F16,
                              kind="ExternalInput"),
            c1=nc.dram_tensor(f"c1_{si}", [128, MT], F32, kind="ExternalInput"),
            w2=nc.dram_tensor(f"w2_{si}", [MT // MJC, 128, MJC, D], F16,
                              kind="ExternalInput"),
            b2=nc.dram_tensor(f"b2_{si}", [128, KT], F32, kind="ExternalInput"),
        ))
    out_x = nc.dram_tensor("out_x", [L_RUN, 128, KT, TOK], F32, kind="ExternalOutput")

    with tile.TileContext(nc) as tc:
        with (
            tc.tile_pool(name="cst", bufs=1) as cst,
            tc.tile_pool(name="wt", bufs=1) as wt,
            tc.tile_pool(name="wts", bufs=2) as wts,
            tc.tile_pool(name="stream", bufs=3) as stream,
            tc.tile_pool(name="y1p", bufs=4) as y1p,
            tc.tile_pool(name="act", bufs=2) as actp,
            tc.tile_pool(name="actx", bufs=2) as actx,
            tc.tile_pool(name="act1", bufs=1) as act1,
            tc.tile_pool(name="state", bufs=1) as state,
            tc.tile_pool(name="tmp", bufs=2) as tmp,
            tc.tile_pool(name="tmp1", bufs=1) as tmp1,
            tc.tile_pool(name="psA", bufs=5, space="PSUM") as psA,
            tc.tile_pool(name="psY", bufs=3, space="PSUM") as psY,
            tc.tile_pool(name="dram", bufs=4, space="DRAM") as dram,
        ):
            cx = Ctx()
            cx.wt, cx.wts, cx.stream, cx.y1p = wt, wts, stream, y1p
            cx.act, cx.actx, cx.act1 = actp, actx, act1
            cx.state, cx.tmp, cx.tmp1 = state, tmp, tmp1
            cx.psA, cx.psY, cx.dram = psA, psY, dram

            cx.onesB = cst.tile([128, 128], BF16, name="onesB")
            nc.vector.memset(cx.onesB[:], 1.0)
            cx.ones1 = cst.tile([1, TOK], F16, name="ones1")
            nc.vector.memset(cx.ones1[:], 1.0)
            cx.epsc = cst.tile([128, 1], F32, name="epsc")
            nc.vector.memset(cx.epsc[:], EPS)
            # block-diag kv holder: off-diagonal blocks stay zero forever
            cx.bd16 = state.tile([128, KT, 128], F16, name="bd16", tag="bd16")
            nc.vector.memset(cx.bd16[:], 0.0)

            x1_sc = dram.tile([L_RUN, 128, KT, TOK], F32, name="x1_sc", tag="x1_sc")
            yf_sc = dram.tile([L_RUN, 128, KT, TOK], F32, name="yf_sc", tag="yf_sc")

            pend = None
            for layer in range(LAYERS_RUN):
                x_src = x_in.ap() if layer == 0 else x1_sc
                last_layer = layer == LAYERS_RUN - 1
                for dir_i, d in enumerate(DIRS_RUN):
                    si = layer * len(DIRS_RUN) + dir_i
                    fwd = d == 0
                    last_scan = dir_i == len(DIRS_RUN) - 1
                    frames = (list(range(L_RUN)) if fwd
                              else list(range(L_RUN - 1, -1, -1)))
                    if not last_scan:
                        out_dst = yf_sc
                    elif last_layer:
                        out_dst = out_x.ap()
                    else:
                        out_dst = x1_sc
                    # L0-bwd MLP adds pos into x1 so layer 1 loads it folded
                    add_pos = (out_dst is x1_sc)
                    pend = _emit_scan(nc, cx, segs[si], x_src, h0_in, pos_in,
                                      frames, fwd=fwd, layer=layer,
                                      yf_sc=yf_sc, out_dst=out_dst,
                                      add_pos=add_pos, pend=pend)
            _emit_mlp(nc, cx, pend)
    nc.compile()
    return nc


def _emit_scan(nc, cx, seg, x_src, h0_in, pos_in, frames, fwd, layer,
               yf_sc, out_dst, add_pos, pend):
    w = {}
    for nm, shape, dt in (("gqkv", [128, KT, F3], F16),
                          ("gqkvh", [128, KT, F3], F16),
                          ("wout", [128, KT, D], F16),
                          ("cqkv", [1, F3], F16)):
        w[nm] = cx.wt.tile(shape, dt, name=nm, tag=nm)
        nc.sync.dma_start(w[nm][:], seg[nm].ap())
    for nm, shape in (("bout", [128, KT]), ("c1", [128, MT]), ("b2", [128, KT])):
        w[nm] = cx.wts.tile(shape, F32, name=nm, tag=nm)
        nc.sync.dma_start(w[nm][:], seg[nm].ap())

    # h carry (f32), re-initialized from h0 each scan
    h32 = cx.state.tile([128, KT, TOK], F32, name="h32", tag="h32")
    nc.sync.dma_start(h32[:], h0_in.ap())

    # fwd: h gets pos[layer] every frame (fixed); bwd: pos[t] per frame
    post_h = None
    if fwd:
        post_h = cx.act1.tile([128, KT, TOK], BF16, name="post_h", tag="post_h")
        nc.sync.dma_start(post_h[:], pos_in.ap()[layer])

    for t in frames:
        pend = _emit_frame(nc, cx, seg, w, t, x_src, h32, pos_in, post_h,
                           yf_sc, fwd, out_dst, add_pos, pend)
    return pend


def _emit_frame(nc, cx, seg, w, t, x_src, h32, pos_in, post_h, yf_sc, fwd,
                out_dst, add_pos, pend):
    # segment-crossing MLP must be emitted before this frame's x DMA
    # (x1_sc RAW follows emission order); in-segment MLP goes at the end
    # so this frame's critical ops outrank it in scheduler priority.
    if pend is not None and pend["w"] is not w:
        _emit_mlp(nc, cx, pend)
        pend = None

    # ---- load x_t (pos already folded in); h_eff = h + pos
    xeff = cx.actx.tile([128, KT, TOK], F32, name="xe", tag="xe")
    nc.sync.dma_start(xeff[:], x_src[t])
    if fwd:
        postile = post_h
    else:
        postile = cx.actx.tile([128, KT, TOK], BF16, name="post", tag="post")
        nc.sync.dma_start(postile[:], pos_in.ap()[t])
    heff = cx.act1.tile([128, KT, TOK], F32, name="heff", tag="heff")
    nc.vector.tensor_add(heff[:], h32[:], postile[:])

    # ---- layer norms + normalized activations (fp16)
    rb_h, mrb_h = _layer_norm(nc, cx, heff, "h")
    zh = _normalize(nc, cx, cx.act1, heff, rb_h, mrb_h, "h")
    rb_x, mrb_x = _layer_norm(nc, cx, xeff, "x")
    zx = _normalize(nc, cx, cx.actx, xeff, rb_x, mrb_x, "x")

    # ---- k, v (token-major): (128, 2, D) each [tok-half, feature]
    # three 512-wide chunks over the 1536 k|v columns per token-half
    k16 = cx.act1.tile([128, 2, D], F16, name="k16", tag="k16")
    v16 = cx.act1.tile([128, 2, D], F16, name="v16", tag="v16")
    for tok2 in range(2):
        for ch in range(3):
            lo = D + ch * 512
            ps = cx.psA.tile([128, 2 * TOK], F32, name="ps", tag="ps")
            pc = ps[:, 0:512]
            for kd in range(KT):
                nc.tensor.matmul(pc, zx[:, kd, tok2 * 128:(tok2 + 1) * 128],
                                 w["gqkv"][:, kd, lo:lo + 512],
                                 start=(kd == 0), stop=False)
            for kd in range(KT):
                nc.tensor.matmul(pc, zh[:, kd, tok2 * 128:(tok2 + 1) * 128],
                                 w["gqkvh"][:, kd, lo:lo + 512],
                                 start=False, stop=False)
            nc.tensor.matmul(pc, cx.ones1[:, 0:128],
                             w["cqkv"][:, lo:lo + 512], start=False, stop=True)
            off = ch * 512
            if ch == 0:
                _elu1(nc, cx, pc, k16[:, tok2, 0:512], 512)
            elif ch == 1:
                _elu1(nc, cx, ps[:, 0:256], k16[:, tok2, 512:768], 256)
                nc.scalar.activation(v16[:, tok2, 0:256], ps[:, 256:512], AF.Copy)
            else:
                nc.scalar.activation(v16[:, tok2, 256:768], pc, AF.Copy)

    # ---- kv state per head-pair; two psums of 3 head-pairs each, then
    # pack diag blocks into (128, 384) f16 with strided DVE ops
    kvpack = cx.act1.tile([128, H * 32], F16, name="kvpack", tag="kvpack")
    for g in range(2):
        ps = cx.psA.tile([128, 2 * TOK], F32, name="ps", tag="ps")
        for i in range(3):
            hp = g * 3 + i
            pskv = ps[:, i * 128:(i + 1) * 128]
            for tok2 in range(2):
                nc.tensor.matmul(pskv, k16[:, tok2, hp * 128:(hp + 1) * 128],
                                 v16[:, tok2, hp * 128:(hp + 1) * 128],
                                 start=(tok2 == 0), stop=(tok2 == 1))
        for half in range(2):
            pr = slice(half * 64, half * 64 + 64)
            src = ps[pr, half * 64:384 + half * 64].rearrange(
                "p (i c) -> p i c", i=3)[:, :, 0:64]
            nc.vector.tensor_scalar_mul(
                kvpack[pr, g * 192:(g + 1) * 192].rearrange(
                    "p (i c) -> p i c", i=3),
                src, KVS)

    # ---- all-reduce kv (fp16) within the token-shard group
    arin = cx.dram.tile([128, H * 32], F16, name="arin", tag="arin")
    arout = cx.dram.tile([128, H * 32], F16, name="arout", tag="arout")
    nc.sync.dma_start(arin[:], kvpack[:])
    nc.gpsimd.collective_compute(
        "AllReduce", ALU.add, replica_groups=REPLICA_GROUPS,
        ins=[arin.opt()], outs=[arout.opt()])

    # ---- q (feature-major): consumed only after the AR, so its matmuls
    # naturally cover the all-reduce latency window
    q16 = cx.act1.tile([128, KT, TOK], F16, name="q16", tag="q16")
    for ft in range(KT):
        ps = cx.psA.tile([128, 2 * TOK], F32, name="ps", tag="ps")
        pq = ps[:, 0:TOK]
        for kd in range(KT):
            nc.tensor.matmul(pq, w["gqkv"][:, kd, ft * 128:(ft + 1) * 128],
                             zx[:, kd, :], start=(kd == 0), stop=False)
        for kd in range(KT):
            nc.tensor.matmul(pq, w["gqkvh"][:, kd, ft * 128:(ft + 1) * 128],
                             zh[:, kd, :], start=False, stop=False)
        nc.tensor.matmul(pq, w["cqkv"][:, ft * 128:(ft + 1) * 128],
                         cx.ones1[:], start=False, stop=True)
        _elu1(nc, cx, pq, q16[:, ft, :], TOK)

    kvred = cx.act1.tile([128, H * 32], F16, name="kvred", tag="kvred")
    nc.sync.dma_start(kvred[:], arout[:])

    # ---- block-diag kv; o_s = blockdiag(kv_s) @ q   (carries KVS)
    for half in range(2):
        pr = slice(half * 64, half * 64 + 64)
        nc.vector.tensor_copy(
            cx.bd16[pr, :, half * 64:half * 64 + 64],
            kvred[pr, :].rearrange("p (k c) -> p k c", k=KT))
    o16 = cx.act1.tile([128, KT, TOK], F16, name="o16", tag="o16")
    for hp in range(KT):
        ps = cx.psA.tile([128, 2 * TOK], F32, name="ps", tag="ps")
        nc.tensor.matmul(ps[:, 0:TOK], cx.bd16[:, hp, :], q16[:, hp, :],
                         start=True, stop=True)
        nc.scalar.activation(o16[:, hp, :], ps[:, 0:TOK], AF.Copy)

    # ---- attn out; h_next = attn + heff; x2 = attn + xeff (in place)
    at_all = cx.act1.tile([128, KT, TOK], F32, name="at_all", tag="at_all")
    for ft in range(KT):
        ps = cx.psA.tile([128, 2 * TOK], F32, name="ps", tag="ps")
        pa = ps[:, 0:TOK]
        for hp in range(KT):
            nc.tensor.matmul(pa, w["wout"][:, hp, ft * 128:(ft + 1) * 128],
                             o16[:, hp, :], start=(hp == 0), stop=(hp == KT - 1))
        # attn = ps*256 + bout  (matmul carries KVS)
        nc.scalar.activation(at_all[:, ft, :], pa, AF.Identity,
                             bias=w["bout"][:, ft:ft + 1], scale=KVSI)
    nc.vector.tensor_add(h32[:], at_all[:], heff[:])
    nc.vector.tensor_add(xeff[:], at_all[:], xeff[:])   # xeff becomes x2

    # ---- z2 for the deferred MLP
    rb2, mrb2 = _layer_norm(nc, cx, xeff, "o")
    z2 = _normalize(nc, cx, cx.actx, xeff, rb2, mrb2, "o")

    new_pend = dict(t=t, z2=z2, x232=xeff, fwd=fwd, out_dst=out_dst,
                    yf_sc=yf_sc, add_pos=add_pos, postile=postile,
                    seg=seg, w=w)
    # in-segment MLP of the previous frame: emitted last so this frame's
    # critical chain outranks it; it still fills every PE idle window
    if pend is not None:
        _emit_mlp(nc, cx, pend)
    return new_pend


def _emit_mlp(nc, cx, pend):
    t, z2, x232 = pend["t"], pend["z2"], pend["x232"]
    fwd, out_dst, yf_sc = pend["fwd"], pend["out_dst"], pend["yf_sc"]
    seg, w = pend["seg"], pend["w"]

    # y1 = gelu(z2 @ G1 + c1) per m-tile, immediately consumed by W2
    yps = [cx.psY.tile([128, 2 * TOK], F32, name="psy", tag="psy")
           for _ in range(KT // 2)]
    for c in range(MT // MJC):
        g1s = cx.stream.tile([128, MJC, KT, 128], F16, name="g1s", tag="g1s")
        nc.sync.dma_start(g1s[:], seg["g1"].ap()[c])
        w2s = cx.stream.tile([128, MJC, D], F16, name="w2s", tag="w2s")
        nc.sync.dma_start(w2s[:], seg["w2"].ap()[c])
        for j in range(MJC):
            mj = c * MJC + j
            ps = cx.psA.tile([128, 2 * TOK], F32, name="ps", tag="ps")
            pj = ps[:, 0:TOK]
            for kd in range(KT):
                nc.tensor.matmul(pj, g1s[:, j, kd, :], z2[:, kd, :],
                                 start=(kd == 0), stop=(kd == KT - 1))
            y1s = cx.y1p.tile([128, TOK], F16, name="y1s", tag="y1s")
            nc.scalar.activation(y1s[:], pj, AF.Gelu,
                                 bias=w["c1"][:, mj:mj + 1])
            for ft in range(KT):
                nc.tensor.matmul(
                    yps[ft // 2][:, (ft % 2) * TOK:(ft % 2 + 1) * TOK],
                    w2s[:, j, ft * 128:(ft + 1) * 128], y1s[:],
                    start=(mj == 0), stop=(mj == MT - 1))

    outt = cx.act1.tile([128, KT, TOK], F32, name="outt", tag="outt")
    for ft in range(KT):
        nc.vector.scalar_tensor_tensor(
            outt[:, ft, :],
            yps[ft // 2][:, (ft % 2) * TOK:(ft % 2 + 1) * TOK],
            w["b2"][:, ft:ft + 1], x232[:, ft, :],
            op0=ALU.add, op1=ALU.add)
    if not fwd:
        yf = cx.act1.tile([128, KT, TOK], F32, name="yfld", tag="yfld")
        nc.sync.dma_start(yf[:], yf_sc[t])
        nc.gpsimd.tensor_add(outt[:], outt[:], yf[:])
        if pend["add_pos"]:
            nc.gpsimd.tensor_add(outt[:], outt[:], pend["postile"][:])
    nc.sync.dma_start(out_dst[t], outt[:])


# ---------------------------------------------------------------- entry point

@functools.cache
def _compiled_nc():
    return build_nc()


def kernel(**inputs):
    inputs = {k: np.asarray(v) for k, v in inputs.items()}
    nc = _compiled_nc()
    in_maps = make_in_maps(inputs)
    res = run_bass_kernel_spmd(nc, in_maps, list(range(NCORES)))
    return unshard_output(res.results)


# revision 30
# speedup vs baseline: 1.0947x; 1.0116x over previous
"""Trainium2 Bass kernel for nn_GPTrack2D (dense transformer with linear
attention and a per-frame recurrence over L).

Sharding: batch (2) -> two groups of 4 cores; tokens (1024 -> 256/core)
within each group. Linear attention's k^T v state is all-reduced per frame
within the group (fp16 payload); the all-reduce and all per-frame stalls
hide behind the previous frame's MLP, which is emitted as gap-filler work
after each frame's critical ops (the Tile scheduler is out-of-order per
engine, by emission-order priority).

Precision: residual stream / carry / LN inputs are fp32. LN stats run as
one fused bf16 [x | x^2] ones-matmul per d-tile (free dim 512). rb comes
from the Scalar engine's Dsqrt (0.5/sqrt); the 0.5 is folded into the
qkv/mlp weight scales on the host. pos embeddings are folded into x on
the host (layer 0) and into the layer-0 backward MLP output (layer 1).
"""

import functools

import ml_dtypes
import numpy as np

import concourse.bacc as bacc
import concourse.mybir as mybir
from concourse import tile
from concourse.bass_utils import run_bass_kernel_spmd

F32 = mybir.dt.float32
BF16 = mybir.dt.bfloat16
F16 = mybir.dt.float16
AF = mybir.ActivationFunctionType
ALU = mybir.AluOpType

B, L, N, D, M, H = 2, 12, 1024, 768, 3072, 12
NCORES = 8
GROUP = 4                 # cores per batch group
TOK = N // GROUP          # 256 tokens per core
KT = D // 128             # 6 d-tiles
MT = M // 128             # 24 m-tiles
MJC = 2                   # m-tiles per MLP weight DMA chunk
F3 = 3 * D                # 2304
EPS = 1e-5
KVS = 1.0 / 256.0         # kv-state scale so fp16 holds it comfortably
KVSI = 256.0

# dev-scale knobs (full problem: L_RUN=12, LAYERS_RUN=2, DIRS_RUN=(0, 1))
L_RUN = L
LAYERS_RUN = 2
DIRS_RUN = (0, 1)

REPLICA_GROUPS = [[0, 1, 2, 3], [4, 5, 6, 7]]


# ---------------------------------------------------------------- host prep

def _pack_weights(inputs, dtype=np.float16):
    """Fold LN gains/biases into weights; x2 for the Dsqrt 0.5; tile for DMA."""
    segs = []
    for layer in range(LAYERS_RUN):
        for d in DIRS_RUN:
            gi = np.asarray(inputs["lni_g"][d, layer]); bi = np.asarray(inputs["lni_b"][d, layer])
            gh = np.asarray(inputs["lnh_g"][d, layer]); bh = np.asarray(inputs["lnh_b"][d, layer])
            go = np.asarray(inputs["lno_g"][d, layer]); bo = np.asarray(inputs["lno_b"][d, layer])
            Wqkv = np.asarray(inputs["Wqkv"][d, layer]); bqkv = np.asarray(inputs["bqkv"][d, layer])
            Wqkvh = np.asarray(inputs["Wqkvh"][d, layer]); bqkvh = np.asarray(inputs["bqkvh"][d, layer])
            Wout = np.asarray(inputs["Wout"][d, layer]); bout = np.asarray(inputs["bout"][d, layer])
            W1 = np.asarray(inputs["W1"][d, layer]); b1 = np.asarray(inputs["b1"][d, layer])
            W2 = np.asarray(inputs["W2"][d, layer]); b2 = np.asarray(inputs["b2"][d, layer])

            gqkv = gi[:, None] * Wqkv                      # (D, 3D)
            gqkvh = gh[:, None] * Wqkvh
            cqkv = bi @ Wqkv + bqkv + bh @ Wqkvh + bqkvh   # (3D,)
            g1 = go[:, None] * W1                          # (D, M)
            c1 = bo @ W1 + b1                              # (M,)

            seg = dict(
                # (128, KT, F3): [p, kd, f] = gqkv[kd*128+p, f]
                gqkv=np.ascontiguousarray(
                    gqkv.reshape(KT, 128, F3).transpose(1, 0, 2)).astype(dtype),
                gqkvh=np.ascontiguousarray(
                    gqkvh.reshape(KT, 128, F3).transpose(1, 0, 2)).astype(dtype),
                cqkv=cqkv.reshape(1, F3).astype(dtype),
                wout=np.ascontiguousarray(
                    Wout.reshape(KT, 128, D).transpose(1, 0, 2)).astype(dtype),
                bout=np.ascontiguousarray(
                    bout.reshape(KT, 128).T).astype(np.float32),  # (128, KT)
                # (MT//MJC, 128, MJC, KT, 128):
                #   [c, p, j, kd, f] = g1[kd*128+p, (c*MJC+j)*128+f]
                g1=np.ascontiguousarray(
                    g1.reshape(KT, 128, MT // MJC, MJC, 128)
                    .transpose(2, 1, 3, 0, 4)).astype(dtype),
                c1=np.ascontiguousarray(
                    c1.reshape(MT, 128).T).astype(np.float32),    # (128, MT)
                # (MT//MJC, 128, MJC, D): [c, p, j, f] = W2[(c*MJC+j)*128+p, f]
                w2=np.ascontiguousarray(
                    W2.reshape(MT // MJC, MJC, 128, D)
                    .transpose(0, 2, 1, 3)).astype(dtype),
                b2=np.ascontiguousarray(
                    b2.reshape(KT, 128).T).astype(np.float32),    # (128, KT)
            )
            segs.append(seg)
    return segs


def _feat_major(a, dtype):
    """(..., tok, D) -> (..., 128, KT, tok) tiled feature-major."""
    t = np.moveaxis(np.asarray(a), -1, -2)                # (..., D, tok)
    shp = t.shape[:-2]
    t = t.reshape(shp + (KT, 128, t.shape[-1]))           # (..., KT, 128, tok)
    t = np.moveaxis(t, -3, -2)                            # (..., 128, KT, tok)
    return np.ascontiguousarray(t).astype(dtype)


def make_in_maps(inputs):
    segs = _pack_weights(inputs)
    x = np.asarray(inputs["x"], np.float32)
    tp = np.asarray(inputs["temporal_pos"], np.float32)
    sp = np.asarray(inputs["spatial_pos"], np.float32)
    pos = tp[:, :, None, :] * sp[:, None, :, :]           # (B, L, N, D)
    xp = x[:, :L_RUN] + pos[:, :L_RUN]                    # layer-0 x + pos fold
    in_maps = []
    for core in range(NCORES):
        b = core // GROUP
        s = (core % GROUP) * TOK
        m = {}
        m["x_in"] = _feat_major(xp[b, :, s:s + TOK, :], np.float32)
        m["h0_in"] = _feat_major(
            np.asarray(inputs["hidden"])[b, s:s + TOK, :], np.float32)
        m["pos_in"] = _feat_major(
            pos[b, :L_RUN, s:s + TOK, :], ml_dtypes.bfloat16)
        for si, seg in enumerate(segs):
            for k, v in seg.items():
                m[f"{k}_{si}"] = v
        in_maps.append(m)
    return in_maps


def unshard_output(results):
    """results: per-core dicts with 'out_x' (L_RUN, 128, KT, TOK) f32."""
    out = np.empty((B, L_RUN, N, D), np.float32)
    for core in range(NCORES):
        b = core // GROUP
        s = (core % GROUP) * TOK
        o = np.asarray(results[core]["out_x"])            # (L, 128, KT, TOK)
        o = o.transpose(0, 2, 1, 3).reshape(L_RUN, D, TOK)
        out[b, :, s:s + TOK, :] = np.moveaxis(o, -1, -2)
    return out


# ---------------------------------------------------------------- kernel build

class Ctx:
    """Pools, constants and persistent tiles used during emission."""


def _layer_norm(nc, cx, src32, tag):
    """Feature-major LN stats for an SBUF (128, KT, TOK) f32 tile.

    Stages [x | x^2] into one bf16 (128, KT, 2, TOK) tile (1 copy + 1
    square, full width), then 6 free-512 ones-matmuls give s1|s2 fused.
    Returns (rb, mrb) f32 (128, TOK): z = src*rb - mrb, where rb is
    0.5/std (Dsqrt) -- the 2x is folded into consumer weights.
    """
    xsq = cx.act.tile([128, KT, 2, TOK], BF16, name="xsq", tag="xsq")
    nc.vector.tensor_copy(xsq[:, :, 0, :], src32[:])
    nc.scalar.activation(xsq[:, :, 1, :], src32[:], AF.Square)
    ps = cx.psA.tile([128, 2 * TOK], F32, name="ps", tag="ps")
    for kd in range(KT):
        nc.tensor.matmul(ps[:], cx.onesB[:], xsq[:, kd, :, :],
                         start=(kd == 0), stop=(kd == KT - 1))
    ms = cx.tmp1.tile([128, 2 * TOK], F32, name="ms", tag="ms")
    nc.vector.tensor_scalar_mul(ms[:], ps[:], 1.0 / D)
    mean, m2 = ms[:, 0:TOK], ms[:, TOK:2 * TOK]
    msq = cx.tmp1.tile([128, TOK], F32, name="msq", tag="msq")
    nc.vector.tensor_mul(msq[:], mean, mean)
    ve = cx.tmp1.tile([128, TOK], F32, name="ve", tag="ve")
    nc.vector.tensor_sub(ve[:], m2, msq[:])
    # rb = 1/sqrt(ve+eps) = exp(-0.5*ln(ve+eps)); ln+exp share one ACT table
    lnv = cx.tmp1.tile([128, TOK], F32, name="lnv", tag="lnv")
    nc.scalar.activation(lnv[:], ve[:], AF.Ln, bias=cx.epsc[:])
    rbp = cx.tmp if tag == "x" else cx.tmp1
    rb = rbp.tile([128, TOK], F32, name=f"rb_{tag}", tag=f"rb_{tag}")
    nc.scalar.activation(rb[:], lnv[:], AF.Exp, scale=-0.5)
    mrb = rbp.tile([128, TOK], F32, name=f"mrb_{tag}", tag=f"mrb_{tag}")
    nc.vector.tensor_mul(mrb[:], mean, rb[:])
    return rb, mrb


def _normalize(nc, cx, pool, src32, rb, mrb, tag):
    """z = src*rb - mrb -> fp16 (128, KT, TOK), two full-width DVE ops."""
    z = pool.tile([128, KT, TOK], F16, name=f"z_{tag}", tag=f"z_{tag}")
    rbb = rb[:].unsqueeze(1).broadcast_to([128, KT, TOK])
    mrbb = mrb[:].unsqueeze(1).broadcast_to([128, KT, TOK])
    nc.vector.tensor_mul(z[:], src32[:], rbb)
    nc.vector.tensor_sub(z[:], z[:], mrbb)
    return z


def _elu1(nc, cx, psum_ap, out_ap, ncols):
    """out = elu(psum)+1 = exp(min(x,0)) + max(x,0)."""
    tmin = cx.tmp.tile([128, 512], F32, name="emin", tag="emin")
    texp = cx.tmp.tile([128, 512], F32, name="eexp", tag="eexp")
    nc.vector.tensor_scalar_min(tmin[:, :ncols], psum_ap, 0.0)
    nc.scalar.activation(texp[:, :ncols], tmin[:, :ncols], AF.Exp)
    nc.vector.scalar_tensor_tensor(out_ap, psum_ap, 0.0, texp[:, :ncols],
                                   op0=ALU.max, op1=ALU.add)


def build_nc():
    nc = bacc.Bacc("TRN2", target_bir_lowering=False, debug=False,
                   num_devices=NCORES)

    x_in = nc.dram_tensor("x_in", [L_RUN, 128, KT, TOK], F32, kind="ExternalInput")
    h0_in = nc.dram_tensor("h0_in", [128, KT, TOK], F32, kind="ExternalInput")
    pos_in = nc.dram_tensor("pos_in", [L_RUN, 128, KT, TOK], BF16, kind="ExternalInput")
    nseg = LAYERS_RUN * len(DIRS_RUN)
    segs = []
    for si in range(nseg):
        segs.append(dict(
            gqkv=nc.dram_tensor(f"gqkv_{si}", [128, KT, F3], F16, kind="ExternalInput"),
            gqkvh=nc.dram_tensor(f"gqkvh_{si}", [128, KT, F3], F16, kind="ExternalInput"),
            cqkv=nc.dram_tensor(f"cqkv_{si}", [1, F3], F16, kind="ExternalInput"),
            wout=nc.dram_tensor(f"wout_{si}", [128, KT, D], F16, kind="ExternalInput"),
            bout=nc.dram_tensor(f"bout_{si}", [128, KT], F32, kind="ExternalInput"),
            g1=nc.dram_tensor(f"g1_{si}", [MT // MJC, 128, MJC, KT, 128], F16,
                              kind="ExternalInput"),
            c1=nc.dram_tensor(f"c1_{si}", [128, MT], F32, kind="ExternalInput"),
            w2=nc.dram_tensor(f"w2_{si}", [MT // MJC, 128, MJC, D], F16,
                              kind="ExternalInput"),
            b2=nc.dram_tensor(f"b2_{si}", [128, KT], F32, kind="ExternalInput"),
        ))
    out_x = nc.dram_tensor("out_x", [L_RUN, 128, KT, TOK], F32, kind="ExternalOutput")

    with tile.TileContext(nc) as tc:
        with (
            tc.tile_pool(name="cst", bufs=1) as cst,
            tc.tile_pool(name="wt", bufs=1) as wt,
            tc.tile_pool(name="wts", bufs=2) as wts,
            tc.tile_pool(name="stream", bufs=3) as stream,
            tc.tile_pool(name="y1p", bufs=4) as y1p,
            tc.tile_pool(name="act", bufs=2) as actp,
            tc.tile_pool(name="actx", bufs=2) as actx,
            tc.tile_pool(name="act1", bufs=1) as act1,
            tc.tile_pool(name="state", bufs=1) as state,
            tc.tile_pool(name="tmp", bufs=2) as tmp,
            tc.tile_pool(name="tmp1", bufs=1) as tmp1,
            tc.tile_pool(name="psA", bufs=5, space="PSUM") as psA,
            tc.tile_pool(name="psY", bufs=3, space="PSUM") as psY,
            tc.tile_pool(name="dram", bufs=4, space="DRAM") as dram,
        ):
            cx = Ctx()
            cx.wt, cx.wts, cx.stream, cx.y1p = wt, wts, stream, y1p
            cx.act, cx.actx, cx.act1 = actp, actx, act1
            cx.state, cx.tmp, cx.tmp1 = state, tmp, tmp1
            cx.psA, cx.psY, cx.dram = psA, psY, dram

            cx.onesB = cst.tile([128, 128], BF16, name="onesB")
            nc.vector.memset(cx.onesB[:], 1.0)
            cx.ones1 = cst.tile([1, TOK], F16, name="ones1")
            nc.vector.memset(cx.ones1[:], 1.0)
            cx.epsc = cst.tile([128, 1], F32, name="epsc")
            nc.vector.memset(cx.epsc[:], EPS)
            # block-diag kv holder: off-diagonal blocks stay zero forever
            cx.bd16 = state.tile([128, KT, 128], F16, name="bd16", tag="bd16")
            nc.vector.memset(cx.bd16[:], 0.0)

            x1_sc = dram.tile([L_RUN, 128, KT, TOK], F32, name="x1_sc", tag="x1_sc")
            yf_sc = dram.tile([L_RUN, 128, KT, TOK], F32, name="yf_sc", tag="yf_sc")

            pend = None
            for layer in range(LAYERS_RUN):
                x_src = x_in.ap() if layer == 0 else x1_sc
                last_layer = layer == LAYERS_RUN - 1
                for dir_i, d in enumerate(DIRS_RUN):
                    si = layer * len(DIRS_RUN) + dir_i
                    fwd = d == 0
                    last_scan = dir_i == len(DIRS_RUN) - 1
                    frames = (list(range(L_RUN)) if fwd
                              else list(range(L_RUN - 1, -1, -1)))
                    if not last_scan:
                        out_dst = yf_sc
                    elif last_layer:
                        out_dst = out_x.ap()
                    else:
                        out_dst = x1_sc
                    # L0-bwd MLP adds pos into x1 so layer 1 loads it folded
                    add_pos = (out_dst is x1_sc)
                    pend = _emit_scan(nc, cx, segs[si], x_src, h0_in, pos_in,
                                      frames, fwd=fwd, layer=layer,
                                      yf_sc=yf_sc, out_dst=out_dst,
                                      add_pos=add_pos, pend=pend)
            _emit_mlp(nc, cx, pend)
    nc.compile()
    return nc


def _emit_scan(nc, cx, seg, x_src, h0_in, pos_in, frames, fwd, layer,
               yf_sc, out_dst, add_pos, pend):
    w = {}
    for nm, shape, dt in (("gqkv", [128, KT, F3], F16),
                          ("gqkvh", [128, KT, F3], F16),
                          ("wout", [128, KT, D], F16),
                          ("cqkv", [1, F3], F16)):
        w[nm] = cx.wt.tile(shape, dt, name=nm, tag=nm)
        nc.sync.dma_start(w[nm][:], seg[nm].ap())
    for nm, shape in (("bout", [128, KT]), ("c1", [128, MT]), ("b2", [128, KT])):
        w[nm] = cx.wts.tile(shape, F32, name=nm, tag=nm)
        nc.sync.dma_start(w[nm][:], seg[nm].ap())

    # h carry (f32), re-initialized from h0 each scan
    h32 = cx.state.tile([128, KT, TOK], F32, name="h32", tag="h32")
    nc.sync.dma_start(h32[:], h0_in.ap())

    # fwd: h gets pos[layer] every frame (fixed); bwd: pos[t] per frame
    post_h = None
    if fwd:
        post_h = cx.act1.tile([128, KT, TOK], BF16, name="post_h", tag="post_h")
        nc.sync.dma_start(post_h[:], pos_in.ap()[layer])

    for t in frames:
        pend = _emit_frame(nc, cx, seg, w, t, x_src, h32, pos_in, post_h,
                           yf_sc, fwd, out_dst, add_pos, pend)
    return pend


def _emit_frame(nc, cx, seg, w, t, x_src, h32, pos_in, post_h, yf_sc, fwd,
                out_dst, add_pos, pend):
    # segment-crossing MLP must be emitted before this frame's x DMA
    # (x1_sc RAW follows emission order); in-segment MLP goes at the end
    # so this frame's critical ops outrank it in scheduler priority.
    if pend is not None and pend["w"] is not w:
        _emit_mlp(nc, cx, pend)
        pend = None

    # ---- load x_t (pos already folded in); h_eff = h + pos
    xeff = cx.actx.tile([128, KT, TOK], F32, name="xe", tag="xe")
    nc.sync.dma_start(xeff[:], x_src[t])
    if fwd:
        postile = post_h
    else:
        postile = cx.actx.tile([128, KT, TOK], BF16, name="post", tag="post")
        nc.sync.dma_start(postile[:], pos_in.ap()[t])
    heff = cx.act1.tile([128, KT, TOK], F32, name="heff", tag="heff")
    nc.vector.tensor_add(heff[:], h32[:], postile[:])

    # ---- layer norms + normalized activations (fp16)
    rb_h, mrb_h = _layer_norm(nc, cx, heff, "h")
    zh = _normalize(nc, cx, cx.act1, heff, rb_h, mrb_h, "h")
    rb_x, mrb_x = _layer_norm(nc, cx, xeff, "x")
    zx = _normalize(nc, cx, cx.actx, xeff, rb_x, mrb_x, "x")

    # ---- k, v (token-major): (128, 2, D) each [tok-half, feature]
    # three 512-wide chunks over the 1536 k|v columns per token-half
    k16 = cx.act1.tile([128, 2, D], F16, name="k16", tag="k16")
    v16 = cx.act1.tile([128, 2, D], F16, name="v16", tag="v16")
    for tok2 in range(2):
        for ch in range(3):
            lo = D + ch * 512
            ps = cx.psA.tile([128, 2 * TOK], F32, name="ps", tag="ps")
            pc = ps[:, 0:512]
            for kd in range(KT):
                nc.tensor.matmul(pc, zx[:, kd, tok2 * 128:(tok2 + 1) * 128],
                                 w["gqkv"][:, kd, lo:lo + 512],
                                 start=(kd == 0), stop=False)
            for kd in range(KT):
                nc.tensor.matmul(pc, zh[:, kd, tok2 * 128:(tok2 + 1) * 128],
                                 w["gqkvh"][:, kd, lo:lo + 512],
                                 start=False, stop=False)
            nc.tensor.matmul(pc, cx.ones1[:, 0:128],
                             w["cqkv"][:, lo:lo + 512], start=False, stop=True)
            off = ch * 512
            if ch == 0:
                _elu1(nc, cx, pc, k16[:, tok2, 0:512], 512)
            elif ch == 1:
                _elu1(nc, cx, ps[:, 0:256], k16[:, tok2, 512:768], 256)
                nc.scalar.activation(v16[:, tok2, 0:256], ps[:, 256:512], AF.Copy)
            else:
                nc.scalar.activation(v16[:, tok2, 256:768], pc, AF.Copy)

    # ---- kv state per head-pair; pack diag blocks into (128, 384) f16
    kvpack = cx.act1.tile([128, H * 32], F16, name="kvpack", tag="kvpack")
    for hp in range(KT):
        ps = cx.psA.tile([128, 2 * TOK], F32, name="ps", tag="ps")
        pskv = ps[:, 0:128]
        for tok2 in range(2):
            nc.tensor.matmul(pskv, k16[:, tok2, hp * 128:(hp + 1) * 128],
                             v16[:, tok2, hp * 128:(hp + 1) * 128],
                             start=(tok2 == 0), stop=(tok2 == 1))
        nc.vector.tensor_scalar_mul(kvpack[0:64, hp * 64:(hp + 1) * 64],
                                    pskv[0:64, 0:64], KVS)
        nc.vector.tensor_scalar_mul(kvpack[64:128, hp * 64:(hp + 1) * 64],
                                    pskv[64:128, 64:128], KVS)

    # ---- all-reduce kv (fp16) within the token-shard group
    arin = cx.dram.tile([128, H * 32], F16, name="arin", tag="arin")
    arout = cx.dram.tile([128, H * 32], F16, name="arout", tag="arout")
    nc.sync.dma_start(arin[:], kvpack[:])
    nc.gpsimd.collective_compute(
        "AllReduce", ALU.add, replica_groups=REPLICA_GROUPS,
        ins=[arin.opt()], outs=[arout.opt()])

    # ---- q (feature-major): consumed only after the AR, so its matmuls
    # naturally cover the all-reduce latency window
    q16 = cx.act1.tile([128, KT, TOK], F16, name="q16", tag="q16")
    for ft in range(KT):
        ps = cx.psA.tile([128, 2 * TOK], F32, name="ps", tag="ps")
        pq = ps[:, 0:TOK]
        for kd in range(KT):
            nc.tensor.matmul(pq, w["gqkv"][:, kd, ft * 128:(ft + 1) * 128],
                             zx[:, kd, :], start=(kd == 0), stop=False)
        for kd in range(KT):
            nc.tensor.matmul(pq, w["gqkvh"][:, kd, ft * 128:(ft + 1) * 128],
                             zh[:, kd, :], start=False, stop=False)
        nc.tensor.matmul(pq, w["cqkv"][:, ft * 128:(ft + 1) * 128],
                         cx.ones1[:], start=False, stop=True)
        _elu1(nc, cx, pq, q16[:, ft, :], TOK)

    kvred = cx.act1.tile([128, H * 32], F16, name="kvred", tag="kvred")
    nc.sync.dma_start(kvred[:], arout[:])

    # ---- block-diag kv; o_s = blockdiag(kv_s) @ q   (carries KVS)
    for half in range(2):
        pr = slice(half * 64, half * 64 + 64)
        nc.vector.tensor_copy(
            cx.bd16[pr, :, half * 64:half * 64 + 64],
            kvred[pr, :].rearrange("p (k c) -> p k c", k=KT))
    o16 = cx.act1.tile([128, KT, TOK], F16, name="o16", tag="o16")
    for hp in range(KT):
        ps = cx.psA.tile([128, 2 * TOK], F32, name="ps", tag="ps")
        nc.tensor.matmul(ps[:, 0:TOK], cx.bd16[:, hp, :], q16[:, hp, :],
                         start=True, stop=True)
        nc.scalar.activation(o16[:, hp, :], ps[:, 0:TOK], AF.Copy)

    # ---- attn out; h_next = attn + heff; x2 = attn + xeff (in place)
    at_all = cx.act1.tile([128, KT, TOK], F32, name="at_all", tag="at_all")
    for ft in range(KT):
        ps = cx.psA.tile([128, 2 * TOK], F32, name="ps", tag="ps")
        pa = ps[:, 0:TOK]
        for hp in range(KT):
            nc.tensor.matmul(pa, w["wout"][:, hp, ft * 128:(ft + 1) * 128],
                             o16[:, hp, :], start=(hp == 0), stop=(hp == KT - 1))
        # attn = ps*256 + bout  (matmul carries KVS)
        nc.scalar.activation(at_all[:, ft, :], pa, AF.Identity,
                             bias=w["bout"][:, ft:ft + 1], scale=KVSI)
    nc.vector.tensor_add(h32[:], at_all[:], heff[:])
    nc.vector.tensor_add(xeff[:], at_all[:], xeff[:])   # xeff becomes x2

    # ---- z2 for the deferred MLP
    rb2, mrb2 = _layer_norm(nc, cx, xeff, "o")
    z2 = _normalize(nc, cx, cx.actx, xeff, rb2, mrb2, "o")

    new_pend = dict(t=t, z2=z2, x232=xeff, fwd=fwd, out_dst=out_dst,
                    yf_sc=yf_sc, add_pos=add_pos, postile=postile,
                    seg=seg, w=w)
    # in-segment MLP of the previous frame: emitted last so this frame's
    # critical chain outranks it; it still fills every PE idle window
    if pend is not None:
        _emit_mlp(nc, cx, pend)
    return new_pend


def _emit_mlp(nc, cx, pend):
    t, z2, x232 = pend["t"], pend["z2"], pend["x232"]
    fwd, out_dst, yf_sc = pend["fwd"], pend["out_dst"], pend["yf_sc"]
    seg, w = pend["seg"], pend["w"]

    # y1 = gelu(z2 @ G1 + c1) per m-tile, immediately consumed by W2
    yps = [cx.psY.tile([128, 2 * TOK], F32, name="psy", tag="psy")
           for _ in range(KT // 2)]
    for c in range(MT // MJC):
        g1s = cx.stream.tile([128, MJC, KT, 128], F16, name="g1s", tag="g1s")
        nc.sync.dma_start(g1s[:], seg["g1"].ap()[c])
        w2s = cx.stream.tile([128, MJC, D], F16, name="w2s", tag="w2s")
        nc.sync.dma_start(w2s[:], seg["w2"].ap()[c])
        for j in range(MJC):
            mj = c * MJC + j
            ps = cx.psA.tile([128, 2 * TOK], F32, name="ps", tag="ps")
            pj = ps[:, 0:TOK]
            for kd in range(KT):
                nc.tensor.matmul(pj, g1s[:, j, kd, :], z2[:, kd, :],
                                 start=(kd == 0), stop=(kd == KT - 1))
            y1s = cx.y1p.tile([128, TOK], F16, name="y1s", tag="y1s")
            nc.scalar.activation(y1s[:], pj, AF.Gelu,
                                 bias=w["c1"][:, mj:mj + 1])
            for ft in range(KT):
                nc.tensor.matmul(
                    yps[ft // 2][:, (ft % 2) * TOK:(ft % 2 + 1) * TOK],
                    w2s[:, j, ft * 128:(ft + 1) * 128], y1s[:],
                    start=(mj == 0), stop=(mj == MT - 1))

    outt = cx.act1.tile([128, KT, TOK], F32, name="outt", tag="outt")
    for ft in range(KT):
        nc.vector.scalar_tensor_tensor(
            outt[:, ft, :],
            yps[ft // 2][:, (ft % 2) * TOK:(ft % 2 + 1) * TOK],
            w["b2"][:, ft:ft + 1], x232[:, ft, :],
            op0=ALU.add, op1=ALU.add)
    if not fwd:
        yf = cx.act1.tile([128, KT, TOK], F32, name="yfld", tag="yfld")
        nc.sync.dma_start(yf[:], yf_sc[t])
        nc.vector.tensor_add(outt[:], outt[:], yf[:])
        if pend["add_pos"]:
            nc.vector.tensor_add(outt[:], outt[:], pend["postile"][:])
    nc.sync.dma_start(out_dst[t], outt[:])


# ---------------------------------------------------------------- entry point

@functools.cache
def _compiled_nc():
    return build_nc()


def kernel(**inputs):
    inputs = {k: np.asarray(v) for k, v in inputs.items()}
    nc = _compiled_nc()
    in_maps = make_in_maps(inputs)
    res = run_bass_kernel_spmd(nc, in_maps, list(range(NCORES)))
    return unshard_output(res.results)
